# revision 70
# baseline (speedup 1.0000x reference)
"""Qwen-style GQA full attention (B=2, S=2048, HID=2048, H=16, KVH=8, D=128)
on 8 trn2 NeuronCores — v2: hi-lo fp8 DoubleRow matmuls.

Sharding: tensor-parallel across head groups (core d owns kv-head d and its
two query heads). Each core emits a partial [B*S, HID] via its Wo row block;
the host sums the 8 partials in f32.

Numerics: projections and Wo run as error-compensated fp8 ("hi-lo"): each
operand x is pre-scaled into e4m3's normal range and split x ~= hi + lo
(both e4m3); y = hi*Wh + lo*Wh + hi*Wl (3 DoubleRow matmuls, K=256 each,
the lo*lo term ~0.03% is dropped). This measures *better* than bf16
(0.11% vs 0.23% on the projection GEMM) at 1.33x bf16 matmul throughput
(DoubleRow streams 2 fp8 rows/cycle). Attention (QK/PV/denominator-sum)
stays bf16: with random weights the softmax is diffuse, so per-element
quantization noise in q/k/v/probs reaches the output at full strength
(fp8 there measurably busts the 2e-2 budget).

Performance notes (sim cost model):
  - Matmul engine cost is out_free_size x cycles_per_row: the softmax
    denominator therefore runs as TRANSPOSED ones-matmuls (stationary =
    probs 128-col slice, moving = ones[128,1], out = [128,1]) — near-free
    on the PE vs ~29us for the classic ones-stationary row-sum form. The
    d^T column is rebuilt as a [1,512] row by four bf16 single-column PE
    transposes, staged to SBUF by one ACT copy, and fanned back across
    partitions with gpsimd partition_broadcast (SBUF-side, Pool).
  - PSUM start_tensor_calc poisons its whole 2KB zero region, so each
    accumulation bank gets exactly ONE start=True per lifetime (the
    sum regions + the split diag pv stops rely on this).
  - Phase 2 is software-pipelined: pv/sums trail their qk by 2 k-tiles
    (hides exp latency under a hot ACT queue), each block's normalize/
    gate "endgame" is spliced into the NEXT block's attention, and Wo
    work drains from a queue a few 512-col groups per iteration. Blocks
    run large/small interleaved, ending on a p=7 block.
  - GPSIMD (Pool) is Q7 software and cannot touch PSUM; it carries the
    rope multiplies, RMSNorm partition_all_reduce, and the denominator
    partition_broadcast. Pool/DVE lack divide and Pool lacks
    TensorScalarPtr on hw, so the gate sigmoid is materialized in
    phase 1 (DVE reciprocal -> bf16) and applied as a plain multiply.
  - Bulk tables ride the Pool queue gated behind a dummy read of the
    last chunk-0 hst slab: emitted before their chunk-0 rope consumers
    (program-order dependency tracking), but transferring after the
    startup-critical loads. Late tables stall only the DVE/Pool rope
    chain, never the PE.
  - sigmoid(g) = 1/(1+exp(-g)): ACT exp + DVE add/reciprocal, keeping the
    ACT table set at {Ln, Exp, Copy} (one table load, no thrash).
"""

import numpy as np
import ml_dtypes

import concourse.bass as bass
import concourse.bass_isa as bass_isa
import concourse.tile as tile
from concourse import bacc, mybir
from contextlib import ExitStack

BF16 = ml_dtypes.bfloat16
E4NP = ml_dtypes.float8_e4m3
F32 = mybir.dt.float32
BF = mybir.dt.bfloat16
E4 = mybir.dt.float8e4
AF = mybir.ActivationFunctionType
DR = mybir.MatmulPerfMode.DoubleRow
ALU = mybir.AluOpType


class _Bacc(bacc.Bacc):
    """Pin the combined Ln+Exp activation table set (see module docstring)."""

    def insert_act_table_loads(self):
        import bass_rust as _bass_rust
        from concourse.hw_specs import get_activation_tables
        has_activation = any(
            isinstance(i, mybir.InstActivation)
            for b in self.main_func.blocks
            for i in b.instructions
        )
        if not has_activation:
            return
        items = [
            (nm, fns if nm == "natural_log_exp_and_others" else set())
            for nm, fns in get_activation_tables(self.m.arch).items()
        ]
        _bass_rust.insert_act_table_loads(self, items)


B, S, HID, H, KVH, D = 2, 2048, 2048, 16, 8, 128
G = H // KVH
EPS = 1e-6
SCALE = D ** -0.5
CH = 512
NCORES = 8

SX = 16.0        # hidden-state fp8 pre-scale
SW = 128.0       # weight fp8 pre-scale
SG = 32.0        # gated-output fp8 pre-scale
IXW = 1.0 / (SX * SW)
IGW = 1.0 / (SG * SW)

# packed projection-weight column layout: q0 q1 k v g0 g1 (128 each)
COLS = {"q0": 0, "q1": 128, "k": 256, "v": 384, "g0": 512, "g1": 640}


def build_nc(S_=S):
    HC = HID // 128
    N = B * S_
    SK = S_ // 128
    NP = S_ // 256
    CPB = S_ // CH
    NT = CH // 128
    NKP = HC // 2

    nc = _Bacc(None)

    hsthi_d = nc.dram_tensor("hsthi", [HC, 128, N], E4, kind="ExternalInput")
    hstlo_d = nc.dram_tensor("hstlo", [HC, 128, N], E4, kind="ExternalInput")
    whi_d = nc.dram_tensor("whi", [HC, 128, 768], E4, kind="ExternalInput")
    wlo_d = nc.dram_tensor("wlo", [HC, 128, 768], E4, kind="ExternalInput")
    wohi_d = nc.dram_tensor("wohi", [G, 128, HID], E4, kind="ExternalInput")
    wolo_d = nc.dram_tensor("wolo", [G, 128, HID], E4, kind="ExternalInput")
    cq_d = nc.dram_tensor("cosq", [128, S_], BF, kind="ExternalInput")
    sq_d = nc.dram_tensor("sinq", [128, S_], BF, kind="ExternalInput")
    ck_d = nc.dram_tensor("cosk", [128, S_], BF, kind="ExternalInput")
    sk_d = nc.dram_tensor("sink", [128, S_], BF, kind="ExternalInput")
    tri_d = nc.dram_tensor("tri2", [128, 2, 256], BF, kind="ExternalInput")
    id_d = nc.dram_tensor("ident", [128, 128], BF, kind="ExternalInput")
    out_d = nc.dram_tensor("out", [N, HID], BF, kind="ExternalOutput")

    with tile.TileContext(nc) as tc, ExitStack() as ctx:
        cpool = ctx.enter_context(tc.tile_pool(name="consts", bufs=1))

        whi_s = cpool.tile([128, HC, 768], E4)
        wlo_s = cpool.tile([128, HC, 768], E4)
        wohi_s = cpool.tile([128, G, HID], E4)
        wolo_s = cpool.tile([128, G, HID], E4)
        cq_s = cpool.tile([128, S_], BF)
        sq_s = cpool.tile([128, S_], BF)
        ck_s = cpool.tile([128, S_], BF)
        sk_s = cpool.tile([128, S_], BF)
        tri_s = cpool.tile([128, 2, 256], BF)
        id_s = cpool.tile([128, 128], BF)
        ones_s = cpool.tile([128, 128], BF)
        o1_s = cpool.tile([128, 1], BF)
        ob_s = cpool.tile([1, 128], BF)
        epsb = cpool.tile([128, 1], F32)
        nc.vector.memset(ones_s[:], 1.0)
        nc.vector.memset(o1_s[:], 1.0)
        nc.vector.memset(ob_s[:], 1.0)
        nc.vector.memset(epsb[:], EPS)

        # weight loads interleaved on ACT (the first matmuls need whi+wlo);
        # tables/Wo weights ride the Pool queue (slack early, needed later)
        whi_v = whi_d[:].rearrange("c p f -> p c f")
        wlo_v = wlo_d[:].rearrange("c p f -> p c f")
        nc.scalar.dma_start(whi_s[:, 0:4, :], whi_v[:, 0:4, :])
        nc.scalar.dma_start(wlo_s[:, 0:4, :], wlo_v[:, 0:4, :])

        # persistent activations
        qtb = cpool.tile([128, B, SK, G, 128], BF)   # rope'd+normed q (feat-major)
        ktb = cpool.tile([128, B, SK, 128], BF)      # rope'd+normed k (feat-major)
        vtb = cpool.tile([128, N], BF)               # v feature-major staging
        vb = cpool.tile([128, B, SK, 128], BF)       # v token-major
        sgb = cpool.tile([128, B, SK, G, 128], BF)   # sigmoid(gate), bf16

        # ---------------- phase 1: projections (hi-lo fp8 DR) ----------------
        # Blocks are emitted sequentially (all 24 accumulating matmuls of one
        # output block, then its consumers) so each PSUM bank is freed ~13us
        # before the next chunk needs it, and consumer work spreads evenly.
        with (
            tc.tile_pool(name="hst", bufs=2) as hstp,
            tc.tile_pool(name="qgps", bufs=4, space="PSUM") as qgps,
            tc.tile_pool(name="auxps", bufs=2, space="PSUM") as auxps,
            tc.tile_pool(name="pwork", bufs=3) as pwork,
            tc.tile_pool(name="pw1", bufs=1) as pw1,
        ):
            for b in range(B):
                for cc in range(CPB):
                    t0 = b * S_ + cc * CH
                    p0 = cc * CH
                    ti0 = cc * NT
                    first_chunk = b == 0 and cc == 0
                    hh = hstp.tile([128, HC, CH], E4, tag="hh")
                    hl = hstp.tile([128, HC, CH], E4, tag="hl")
                    if first_chunk:
                        # id first (tiny, V-transpose on the PE needs it at
                        # ~chunk-0 end), then the remaining weight slabs
                        nc.scalar.dma_start(id_s[:], id_d[:])
                        for c4 in range(4, HC, 4):
                            nc.scalar.dma_start(whi_s[:, c4:c4 + 4, :],
                                                whi_v[:, c4:c4 + 4, :])
                            nc.scalar.dma_start(wlo_s[:, c4:c4 + 4, :],
                                                wlo_v[:, c4:c4 + 4, :])
                    step = 4 if first_chunk else 8
                    for c4 in range(0, HC, step):
                        nc.sync.dma_start(
                            hh[:, c4:c4 + step, :],
                            hsthi_d[c4:c4 + step, :, t0:t0 + CH].rearrange(
                                "c p f -> p c f"))
                        nc.sync.dma_start(
                            hl[:, c4:c4 + step, :],
                            hstlo_d[c4:c4 + step, :, t0:t0 + CH].rearrange(
                                "c p f -> p c f"))
                    if first_chunk:
                        # rope tables must be emitted BEFORE their chunk-0
                        # rope consumers (dependency tracking is program-
                        # order), but their transfers are gated behind the
                        # last chunk-0 hst slab by a dummy Pool read so the
                        # startup DMA window stays clear. Late tables only
                        # stall the DVE/Pool rope chain, never the PE.
                        dum = pw1.tile([1, 1], E4, tag="dum")
                        nc.gpsimd.tensor_copy(dum[:], hh[0:1, HC - 1, 0:1])
                        for dst, srct in ((cq_s, cq_d), (sq_s, sq_d),
                                          (ck_s, ck_d), (sk_s, sk_d)):
                            nc.gpsimd.dma_start(dst[:], srct[:])
                    if b == 0 and cc == 2:
                        nc.sync.dma_start(tri_s[:], tri_d[:])
                    if b == 0 and cc == 3:
                        nc.sync.dma_start(
                            wohi_s[:], wohi_d[:].rearrange("c p f -> p c f"))
                    if b == 1 and cc == 0:
                        nc.sync.dma_start(
                            wolo_s[:], wolo_d[:].rearrange("c p f -> p c f"))

                    xus = {}
                    ssts = {}
                    e1 = pw1.tile([128, 2, CH], BF, tag="e1")
                    s12s = {}

                    def run_blocks(specs):
                        # specs: [(nm, n_prods)]; for the DMA-bound first
                        # chunk the blocks advance kp-inner together so
                        # matmuls track ht chunk arrival instead of waiting
                        # for the whole tensor
                        pss, prodss = [], []
                        for nm, n_prods in specs:
                            pss.append(qgps.tile([128, CH], F32, tag="pp",
                                                 name=f"ps_{nm}"))
                            prodss.append(
                                [(whi_s, hh), (whi_s, hl),
                                 (wlo_s, hh)][:n_prods])
                        for kp in range(NKP):
                            c = 2 * kp
                            for (nm, _), ps, prods in zip(specs, pss, prodss):
                                col0 = COLS[nm]
                                for pi, (wsrc, hsrc) in enumerate(prods):
                                    nc.tensor.matmul(
                                        ps[:],
                                        wsrc[:, c:c + 2, col0:col0 + 128],
                                        hsrc[:, c:c + 2, :],
                                        start=kp == 0 and pi == 0,
                                        stop=(kp == NKP - 1
                                              and pi == len(prods) - 1),
                                        perf_mode=DR)
                        return pss

                    def run_block(nm, n_prods):
                        return run_blocks([(nm, n_prods)])[0]

                    def norm_stats(nm, i, ps):
                        xu = pwork.tile([128, CH], BF, tag=f"xu_{nm}",
                                        name="xu")
                        nc.scalar.activation(xu[:], ps[:], AF.Copy, scale=IXW)
                        xus[nm] = xu
                        xsq = pw1.tile([128, CH], BF, tag=f"xsq_{nm}",
                                       name="xsq")
                        nc.vector.tensor_mul(xsq[:], xu[:], xu[:])
                        ssB = pw1.tile([128, CH], F32, tag=f"ssB_{nm}",
                                       name="ssB")
                        nc.gpsimd.partition_all_reduce(
                            ssB[:], xsq[:], 128, bass_isa.ReduceOp.add)
                        lnB = pw1.tile([128, CH], BF, tag=f"lnB_{nm}",
                                       name="lnB")
                        nc.scalar.activation(lnB[:], ssB[:], AF.Ln,
                                             bias=epsb[:], scale=1.0 / D)
                        rstdB = pw1.tile([128, CH], BF, tag=f"rstdB_{nm}",
                                         name="rstdB")
                        nc.scalar.activation(rstdB[:], lnB[:], AF.Exp,
                                             scale=-0.5)
                        ssts[nm] = rstdB

                    def rope_sum(nm, i, ctab, stab, cidx):
                        # 2-input SBUF ops need equal base partitions, so the
                        # half-rotation is a pair of 1-input copies first.
                        xu = xus[nm]
                        t1 = pw1.tile([128, CH], BF, tag=f"t1_{nm}", name="t1")
                        nc.vector.tensor_mul(t1[:], xu[:],
                                             ctab[:, cidx, p0:p0 + CH]
                                             if cidx is not None
                                             else ctab[:, p0:p0 + CH])
                        xr = pw1.tile([128, CH], BF, tag=f"xr_{nm}", name="xr")
                        nc.vector.tensor_copy(xr[0:64, :], xu[64:128, :])
                        nc.vector.tensor_copy(xr[64:128, :], xu[0:64, :])
                        t2 = pw1.tile([128, CH], BF, tag=f"t2_{nm}", name="t2")
                        sv = (stab[:, cidx, p0:p0 + CH] if cidx is not None
                              else stab[:, p0:p0 + CH])
                        nc.gpsimd.tensor_mul(t2[:], xr[:], sv)
                        s12 = pw1.tile([128, CH], BF, tag=f"s12_{nm}",
                                       name="s12")
                        nc.vector.tensor_add(s12[:], t1[:], t2[:])
                        s12s[nm] = s12

                    def emit_bc(nm, s12, dest):
                        nc.vector.scalar_tensor_tensor(dest, s12[:], 1.0,
                                                       ssts[nm][:],
                                                       ALU.mult, ALU.mult)

                    tabs = {"q0": (cq_s, sq_s, None),
                            "q1": (cq_s, sq_s, None),
                            "k": (ck_s, sk_s, None)}
                    names3 = ("q0", "q1", "k")

                    dests = {"q0": qtb[:, b, ti0:ti0 + NT, 0, :],
                             "q1": qtb[:, b, ti0:ti0 + NT, 1, :],
                             "k": ktb[:, b, ti0:ti0 + NT, :]}

                    def qcons(nm):
                        i = names3.index(nm)
                        norm_stats(nm, i, pss[nm])
                        rope_sum(nm, i, *tabs[nm])
                        emit_bc(nm, s12s[nm], dests[nm])

                    def vcons():
                        nc.scalar.activation(vtb[:, t0:t0 + CH], pss["v"][:],
                                             AF.Copy, scale=IXW)

                    def gcons(h):
                        nc.scalar.activation(e1[:, h, :],
                                             pss["g0" if h == 0 else "g1"][:],
                                             AF.Exp, scale=-IXW)

                    pss = {}
                    if first_chunk:
                        # DMA-bound: advance pairs of blocks kp-inner so the
                        # PE tracks ht chunk arrival
                        for grp in ([("q0", 3), ("q1", 3)],
                                    [("k", 3), ("v", 3)],
                                    [("g0", 2), ("g1", 2)]):
                            res = run_blocks(grp)
                            pss.update({nm: ps for (nm, _), ps
                                        in zip(grp, res)})
                            if grp[0][0] == "q0":
                                qcons("q0")
                                qcons("q1")
                            elif grp[0][0] == "k":
                                qcons("k")
                                vcons()
                            else:
                                gcons(0)
                                gcons(1)
                    else:
                        for nm in names3:
                            pss[nm] = run_block(nm, 3)
                            qcons(nm)
                        pss["v"] = run_block("v", 3)
                        vcons()
                        pss["g0"] = run_block("g0", 2)
                        gcons(0)
                        pss["g1"] = run_block("g1", 2)
                        gcons(1)

                    # this chunk's V tiles -> token-major (PE transposes)
                    vt_ps = auxps.tile([128, 512], BF, tag="aux", name="vt")
                    for jj in range(NT):
                        j = ti0 + jj
                        nc.tensor.transpose(
                            vt_ps[:, jj * 128:(jj + 1) * 128],
                            vtb[:, b * S_ + j * 128:b * S_ + (j + 1) * 128],
                            id_s[:])
                    nc.vector.tensor_copy(vb[:, b, ti0:ti0 + NT, :], vt_ps[:])

                    # sigmoid = 1/(1+e1), stored bf16 so the phase-2 gate
                    # multiply runs at DVE 2x (or on Pool)
                    a1f = pw1.tile([128, 2, CH], F32, tag="a1f")
                    nc.vector.tensor_scalar_add(a1f[:], e1[:], 1.0)
                    with nc.allow_low_precision(reason="sigmoid to bf16"):
                        for h in range(G):
                            nc.vector.reciprocal(
                                sgb[:, b, ti0:ti0 + NT, h, :], a1f[:, h, :])


        # ---------------- phase 2: attention + gating + Wo ----------------
        # PSUM banks: scores 2x[2-bank] + pv 1 + wo 2 + rbank 1 = 8.
        # The softmax denominator never runs as wide ones-matmuls: per probs
        # tile it is 8 transposed matmuls with out free-size 1 (engine-free
        # per the cost model) accumulating d^T[col,1] into rbank cols 448-451,
        # then per block: reciprocal -> PE transpose -> partition-spread copy
        # -> 4 tiny [1,128] broadcast matmuls rebuild 1/d as [128,512] in
        # rbank. ~24us of PE engine time cheaper than the ones-matmul scheme.
        with (
            tc.tile_pool(name="scps", bufs=2, space="PSUM") as scps,
            tc.tile_pool(name="pvps", bufs=1, space="PSUM") as pvps,
            tc.tile_pool(name="rbps", bufs=1, space="PSUM") as rbps,
            tc.tile_pool(name="wops", bufs=2, space="PSUM") as wops,
            tc.tile_pool(name="probsp", bufs=4) as probsp,
            tc.tile_pool(name="awork", bufs=3) as awork,
        ):
            # Wo groups (one per 512-col psum) are queued and drained a
            # couple per attention iteration so wo work never bunches up.
            # Copies: 2/8 on ACT (the exp engine), 6/8 on DVE.
            wo_q = []
            wo_state = {}

            FINAL_I0 = None  # set below once ORDER1 is known

            def wo_emit_one():
                b_, i0_, ghi, glo, it, oc = wo_q.pop(0)
                final = b_ == B - 1 and i0_ == FINAL_I0
                key = (b_, i0_)
                if key not in wo_state:
                    wo_state[key] = [
                        awork.tile([128, 2, HID], BF, tag="osb", name="osb"),
                        0, 0]
                st = wo_state[key]
                wop = wops.tile([128, 512], F32, tag="wo")
                for pi, (gs, ws) in enumerate(
                        ((ghi, wohi_s), (glo, wohi_s), (ghi, wolo_s))):
                    nc.tensor.matmul(
                        wop[:], gs[:, it, :, :],
                        ws[:, :, oc * 512:(oc + 1) * 512],
                        start=pi == 0, stop=pi == 2, perf_mode=DR)
                dst = st[0][:, it, oc * 512:(oc + 1) * 512]
                if (oc % 2 == 0) if final else (oc == 0):
                    nc.scalar.activation(dst, wop[:], AF.Copy, scale=IGW)
                else:
                    nc.vector.tensor_scalar_mul(dst, wop[:], IGW)
                st[1 + it] += 1
                if final:
                    # tail: store each 512-col slab as soon as it is staged
                    trow = b_ * S_ + (i0_ + it) * 128
                    nc.sync.dma_start(
                        out_d[trow:trow + 128, oc * 512:(oc + 1) * 512], dst)
                elif st[1 + it] == 4:
                    trow = b_ * S_ + (i0_ + it) * 128
                    q = nc.sync if it == 0 else nc.gpsimd
                    q.dma_start(out_d[trow:trow + 128, :], st[0][:, it, :])

            def wo_drain(n):
                for _ in range(min(n, len(wo_q))):
                    wo_emit_one()

            deferred = []
            hilo = {}
            # large/small interleave, ending on the biggest block so the
            # deferred wo stores always have a fat block to hide under
            ORDER0 = [7, 0, 6, 1, 5, 2, 4, 3]
            ORDER1 = [3, 4, 2, 5, 1, 6, 0, 7]
            FINAL_I0 = 2 * ORDER1[-1]
            for b in range(B):
                for p in (ORDER0 if b == 0 else ORDER1):
                    i0, i1 = 2 * p, 2 * p + 1
                    last_block = b == B - 1 and p == ORDER1[-1]
                    pv = pvps.tile([128, 512], F32, tag="pv", name="pv")
                    rbank = rbps.tile([128, 512], F32, tag="rb", name="rbank")
                    mvq = qtb[:, b, i0:i0 + 2, :, :]
                    splice = deferred
                    deferred = []

                    def qk_sub(t, sub, scp):
                        j = 2 * t + sub
                        if t < p or sub == 0:
                            nc.tensor.matmul(scp[:, sub, :], ktb[:, b, j, :],
                                             mvq)
                        else:
                            nc.tensor.matmul(scp[:, 1, 0:256],
                                             ktb[:, b, j, :],
                                             qtb[:, b, i1, :, :])

                    def exp_emit(t, scp, probs):
                        if t < p:
                            nc.scalar.activation(probs[:], scp[:], AF.Exp)
                        else:
                            nc.scalar.activation(probs[:, 0, :], scp[:, 0, :],
                                                 AF.Exp)
                            nc.scalar.activation(probs[:, 1, 0:256],
                                                 scp[:, 1, 0:256], AF.Exp)

                    def pv_emit(t, probs):
                        j0, j1 = 2 * t, 2 * t + 1
                        first = t == 0
                        if t < p:
                            for sub, j in ((0, j0), (1, j1)):
                                nc.tensor.matmul(
                                    pv[:], vb[:, b, j, :], probs[:, sub, :],
                                    start=first and sub == 0, stop=False)
                        else:
                            # single region-closing stop on the last matmul
                            # (stop flags are sim bookkeeping; hw ignores
                            # them). For p==0 the second start=True would
                            # trip the sim's one-start-per-region check, so
                            # it alone skips it.
                            nc.tensor.matmul(pv[:, 0:256], vb[:, b, j0, :],
                                             probs[:, 0, 0:256],
                                             start=first, stop=False)
                            nc.tensor.matmul(pv[:, 256:512], vb[:, b, j0, :],
                                             probs[:, 0, 256:512],
                                             start=first, stop=False,
                                             skip_group_check=first)
                            nc.tensor.matmul(pv[:, 256:512], vb[:, b, j1, :],
                                             probs[:, 1, 0:256],
                                             start=False, stop=True)

                    def sum_emit(t, probs):
                        # transposed denominator: out free-size 1 => ~free on
                        # the PE engine; region c accumulates q-cols
                        # [128c,128c+128) of this block in rbank col 448+c.
                        # start=True poisons the WHOLE 2KB zero region
                        # (pending-zero marks cover the bank), so exactly ONE
                        # start per block: the very first matmul. Later
                        # regions' first writes accumulate onto pending-zero
                        # bytes, which read as zero.
                        if t < p:
                            for sub in (0, 1):
                                for c in range(4):
                                    nc.tensor.matmul(
                                        rbank[:, 448 + c:449 + c],
                                        probs[:, sub, c * 128:(c + 1) * 128],
                                        ones_s[:, 0:1],
                                        start=t == 0 and sub == 0 and c == 0,
                                        stop=False,
                                        skip_group_check=True)
                        else:
                            for c in range(4):
                                nc.tensor.matmul(
                                    rbank[:, 448 + c:449 + c],
                                    probs[:, 0, c * 128:(c + 1) * 128],
                                    ones_s[:, 0:1],
                                    start=t == 0 and c == 0, stop=c < 2,
                                    skip_group_check=True)
                            for c in range(2):
                                nc.tensor.matmul(
                                    rbank[:, 450 + c:451 + c],
                                    probs[:, 1, c * 128:(c + 1) * 128],
                                    ones_s[:, 0:1],
                                    start=False, stop=True,
                                    skip_group_check=True)

                    def endgame(pv=pv, rbank=rbank, b=b, i0=i0,
                                last_block=last_block):
                        # denominator: d^T -> 1/d -> transpose -> spread bf16
                        # -> broadcast back to [128,512] f32 (rbank reuse),
                        # then normalize+gate+hi-lo split. Runs spliced into
                        # the NEXT block so the PE<->DVE ping-pong overlaps
                        # that block's attention matmuls.
                        dTs = awork.tile([128, 4], BF, tag="dTs")
                        with nc.allow_low_precision(reason="1/d to bf16"):
                            nc.vector.reciprocal(dTs[:], rbank[:, 448:452])
                        # four single-column bf16 transposes (the hw-proven
                        # transpose path) lay 1/d out as one [1,512] row in
                        # a borrowed wo-pool slot, then one ACT copy stages
                        # it in SBUF and a gpsimd partition_broadcast fans
                        # it back out across all 128 partitions
                        rowt = scps.tile([1, 512], BF, tag="sc", name="rowt")
                        for c in range(4):
                            nc.tensor.matmul(
                                rowt[:, c * 128:(c + 1) * 128],
                                dTs[:, c:c + 1], id_s[:],
                                is_transpose=True, skip_group_check=True)
                        dsp = awork.tile([1, 512], BF, tag="dsp")
                        nc.scalar.activation(dsp[:], rowt[:], AF.Copy)
                        rsb = awork.tile([128, 512], BF, tag="rsb")
                        nc.gpsimd.partition_broadcast(rsb[:], dsp[:])
                        tmp = awork.tile([128, 512], BF, tag="tmp")
                        nc.vector.tensor_mul(tmp[:], pv[:], rsb[:])
                        gfull = awork.tile([128, 2, 2, 128], BF, tag="gf")
                        nc.vector.tensor_mul(gfull[:], tmp[:],
                                             sgb[:, b, i0:i0 + 2, :, :])
                        ghi = probsp.tile([128, 2, 2, 128], E4, tag="ghi")
                        nc.vector.tensor_scalar_mul(ghi[:], gfull[:], SG)
                        glo = probsp.tile([128, 2, 2, 128], E4, tag="glo")
                        nc.vector.scalar_tensor_tensor(glo[:], gfull[:], SG,
                                                       ghi[:], ALU.mult,
                                                       ALU.subtract)
                        for it in (0, 1):
                            for oc in range(4):
                                wo_q.append((b, i0, ghi, glo, it, oc))

                    # pipeline: qk(t)-sub0 | pv+sums(t-2) | qk(t)-sub1, so
                    # the exp latency is fully hidden even when the ACT
                    # queue runs hot. splice[0] (previous block's endgame)
                    # must precede this block's first sum_emit (rbank reuse).
                    hist = []
                    si = [0]

                    def run_splice(k):
                        while si[0] < min(k, len(splice)):
                            splice[si[0]]()
                            si[0] += 1

                    for t in range(p + 1):
                        scp_cur = scps.tile([128, 2, 512], F32, tag="sc",
                                            name="scp")
                        probs_cur = probsp.tile([128, 2, 512], BF,
                                                tag="probs", name="probs")
                        qk_sub(t, 0, scp_cur)
                        if len(hist) >= 2:
                            pv_emit(*hist[-2])
                            sum_emit(*hist[-2])
                        qk_sub(t, 1, scp_cur)
                        exp_emit(t, scp_cur, probs_cur)
                        if t == p:
                            nc.vector.tensor_mul(probs_cur[:, :, 0:256],
                                                 probs_cur[:, :, 0:256],
                                                 tri_s[:])
                        if t == 1:
                            run_splice(1)
                        if t >= 2:
                            wo_drain(3)
                        hist.append((t, probs_cur))
                    run_splice(1)
                    for tt_pr in hist[-2:]:
                        pv_emit(*tt_pr)
                        sum_emit(*tt_pr)
                    deferred = [endgame]
            deferred[0]()
            wo_drain(len(wo_q))
    nc.compile()
    return nc


def prep_inputs(hidden_states, cos, sin, Wq, Wk, Wv, Wo, q_norm_w, k_norm_w,
                S_=S):
    N = B * S_
    hsT = np.ascontiguousarray(
        hidden_states.reshape(N, HID).T).astype(np.float32) * SX
    hsthi = hsT.astype(E4NP)
    hstlo = (hsT - hsthi.astype(np.float32)).astype(E4NP)
    HC = HID // 128
    hsthi = hsthi.reshape(HC, 128, N)
    hstlo = hstlo.reshape(HC, 128, N)

    cos0 = np.asarray(cos[0], np.float32)
    sin0 = np.asarray(sin[0], np.float32)
    qw = np.asarray(q_norm_w, np.float32)
    kw = np.asarray(k_norm_w, np.float32)
    sign = np.where(np.arange(D) < 64, -1.0, 1.0).astype(np.float32)
    shift = (np.arange(D) + 64) % D

    cosq = np.ascontiguousarray(cos0.T * qw[:, None] * SCALE).astype(BF16)
    sinq = np.ascontiguousarray(
        sin0.T * (sign * qw[shift])[:, None] * SCALE).astype(BF16)
    cosk = np.ascontiguousarray(cos0.T * kw[:, None]).astype(BF16)
    sink = np.ascontiguousarray(
        sin0.T * (sign * kw[shift])[:, None]).astype(BF16)

    # diag mask: probs[:, sub, 0:256] has k-token on partitions and
    # (head, tok) on columns; keep k <= q i.e. p <= col % 128
    toks = np.arange(256) % 128
    tri2 = np.ascontiguousarray(np.stack(
        [(np.arange(128)[:, None] <= toks[None, :]).astype(BF16)] * 2, axis=1))
    ident = np.eye(128, dtype=BF16)

    in_maps = []
    for d in range(NCORES):
        h0, h1 = G * d, G * d + 1
        cols = [Wq[:, h0 * 2 * D: h0 * 2 * D + D],
                Wq[:, h1 * 2 * D: h1 * 2 * D + D],
                Wk[:, d * D:(d + 1) * D],
                Wv[:, d * D:(d + 1) * D],
                Wq[:, h0 * 2 * D + D: (h0 + 1) * 2 * D],
                Wq[:, h1 * 2 * D + D: (h1 + 1) * 2 * D]]
        wcols = np.concatenate(cols, axis=1).astype(np.float32) * SW
        whi = wcols.astype(E4NP)
        wlo = (wcols - whi.astype(np.float32)).astype(E4NP)

        wo_rows = np.ascontiguousarray(
            Wo[d * G * D:(d + 1) * G * D, :]).astype(np.float32) * SW
        wohi = wo_rows.astype(E4NP)
        wolo = (wo_rows - wohi.astype(np.float32)).astype(E4NP)

        in_maps.append({
            "hsthi": hsthi, "hstlo": hstlo,
            "whi": np.ascontiguousarray(whi).reshape(HC, 128, 768),
            "wlo": np.ascontiguousarray(wlo).reshape(HC, 128, 768),
            "wohi": wohi.reshape(G, 128, HID),
            "wolo": wolo.reshape(G, 128, HID),
            "cosq": cosq, "sinq": sinq, "cosk": cosk, "sink": sink,
            "tri2": tri2, "ident": ident,
        })
    return in_maps


_NC_CACHE = {}
_RUNNER_CACHE = {}


def _get_nc(S_=S):
    if S_ not in _NC_CACHE:
        _NC_CACHE[S_] = build_nc(S_)
    return _NC_CACHE[S_]


def _get_runner(S_=S):
    if S_ in _RUNNER_CACHE:
        return _RUNNER_CACHE[S_]
    import jax
    from jax.experimental.shard_map import shard_map
    from jax.sharding import Mesh, PartitionSpec
    from concourse import bass2jax, mybir as _mybir
    bass2jax.install_neuronx_cc_hook()

    nc = _get_nc(S_)
    assert nc.dbg_addr is None
    pid_name = (nc.partition_id_tensor.name
                if nc.partition_id_tensor is not None else None)

    in_names, out_names, out_avals = [], [], []
    for alloc in nc.m.functions[0].allocations:
        if not isinstance(alloc, _mybir.MemoryLocationSet):
            continue
        name = alloc.memorylocations[0].name
        if alloc.kind == "ExternalInput":
            if name != pid_name:
                in_names.append(name)
        elif alloc.kind == "ExternalOutput":
            out_names.append(name)
            out_avals.append(jax.core.ShapedArray(
                tuple(alloc.tensor_shape), _mybir.dt.np(alloc.dtype)))
    n_params = len(in_names)
    all_names = in_names + out_names
    if pid_name is not None:
        all_names = all_names + [pid_name]

    def _body(*args):
        operands = list(args)
        if pid_name is not None:
            operands.append(bass2jax.partition_id_tensor())
        outs = bass2jax._bass_exec_p.bind(
            *operands,
            out_avals=tuple(out_avals),
            in_names=tuple(all_names),
            out_names=tuple(out_names),
            lowering_input_output_aliases=(),
            sim_require_finite=True,
            sim_require_nnan=True,
            nc=nc,
        )
        return tuple(outs)

    devices = jax.devices()[:NCORES]
    mesh = Mesh(np.asarray(devices), ("core",))
    nin = n_params + len(out_names)
    sharded = jax.jit(
        shard_map(_body, mesh=mesh,
                  in_specs=(PartitionSpec("core"),) * nin,
                  out_specs=(PartitionSpec("core"),) * len(out_names),
                  check_rep=False),
        keep_unused=True,
    )
    zeros = [np.zeros((NCORES * a.shape[0], *a.shape[1:]), a.dtype)
             for a in out_avals]
    zeros_dev = [jax.device_put(z) for z in zeros]

    def run(in_maps):
        concat_in = [
            np.concatenate([np.asarray(m[nm]) for m in in_maps], axis=0)
            for nm in in_names
        ]
        outs = sharded(*concat_in, *zeros_dev)
        return {nm: np.asarray(outs[i]) for i, nm in enumerate(out_names)}

    def run_prepared(dev_args):
        return sharded(*dev_args, *zeros_dev)

    def prepare(in_maps):
        return [
            jax.device_put(np.concatenate(
                [np.asarray(m[nm]) for m in in_maps], axis=0))
            for nm in in_names
        ]

    r = {"run": run, "prepare": prepare, "run_prepared": run_prepared,
         "out_names": out_names, "out_avals": out_avals}
    _RUNNER_CACHE[S_] = r
    return r


def kernel(hidden_states, cos, sin, Wq, Wk, Wv, Wo, q_norm_w, k_norm_w):
    in_maps = prep_inputs(hidden_states, cos, sin, Wq, Wk, Wv, Wo,
                          q_norm_w, k_norm_w)
    runner = _get_runner()
    outs = runner["run"](in_maps)
    full = outs["out"].reshape(NCORES, B * S, HID)
    acc = full.astype(np.float32).sum(axis=0)
    return acc.reshape(B, S, HID)

